# revision 1
# baseline (speedup 1.0000x reference)
"""Trainium2 Bass kernel for a cross-attention block.

Problem (hardcoded shapes): B=4, S=2048, T=256, H=2048, NH=16, HD=128.
  q = hs @ Wq.T + bq ; k = at @ Wk.T + bk ; v = at @ Wv.T + bv   (per-head 128-dim)
  scores = q k^T / sqrt(128), masked over audio positions, softmax over T
  out = LN(clip(rs) * (ctx @ Wo.T + bo)) * gamma + beta

Sharding: pure data parallel over (batch, S/2) -> 8 shards, one per NeuronCore.
Each core gets host-pretransposed bf16 operands so the whole on-device pipeline
needs zero transposes:
  - hsT [H, Sc], atT [H, T], WqT/WkT/WvT/WoT = W.T [H, H]
  - K^T  [d, t]  = sum_c WkT[c, d-tile] x atT[c, t]         (lhsT=WkT, rhs=atT)
  - V    [t, d]  = sum_c atT[c, t-tile] x WvT[c, d]         (lhsT=atT, rhs=WvT)
  - Q^T  [d, s]  = sum_c WqT[c, d-tile] x hsT[c, s]         (lhsT=WqT, rhs=hsT)
  - S^T  [t, s]  = K^T-tile.T x Q^T   per head              (lhsT=K^T,  rhs=Q^T)
  - expT [t, s]  = Exp(scale * S^T + mask_bias[t])          (ACT, mask as bias)
  - den  [1, s]  = ones[t,1].T x expT                       (PE column-sum)
  - rcp  [1, s]  = 1/den (DVE), bcast [128, s] via K=1 matmul with ones[1,128]
  - ctxT [d, s]  = V-tile.T x expT, then ctxT *= bcast (fused in PSUM->SBUF copy)
  - delta[s, e]  = sum_c ctxT[c, s-tile].T x WoT[c, e]      (lhsT=ctxT, rhs=WoT)
  - LN over e (free axis) via bn_stats/bn_aggr; residual_scale folded into rstd:
      out = (delta - mu) / sqrt(var + eps/rs^2)   [* gamma + beta if nontrivial]
"""

import math
import os
import sys

import numpy as np

for _p in ("/opt/trn_rl_repo", "/root/.axon_site/_ro/trn_rl_repo"):
    if os.path.isdir(_p) and _p not in sys.path:
        sys.path.insert(0, _p)

import ml_dtypes

import concourse.bass as bass
import concourse.mybir as mybir
import concourse.tile as tile
from concourse import bacc

BF16 = mybir.dt.bfloat16
F32 = mybir.dt.float32
AF = mybir.ActivationFunctionType
ALU = mybir.AluOpType

P = 128
MASK_NEG = -100.0  # additive bias for masked keys; exp(-100+~6) == 0 in fp32
EPS_LN = 1e-5
RES_SCALE_MAX = 0.3


def _bcast_row_ap(ap_1d, rows):
    """DRAM [N] -> broadcast AP [rows, N] (partition stride 0)."""
    return bass.AP(tensor=ap_1d.tensor, offset=ap_1d.offset,
                   ap=[[0, rows], list(ap_1d.ap[0])])


def emit_cross_attn(tc, io, S, T, H, NH, rs, use_qkv_bias, use_gamma_beta):
    """Emit the full per-core pipeline. io maps name -> DRAM AP."""
    nc = tc.nc
    C = H // P            # contraction chunks (== NH when HD==128)
    NT = T // P           # t chunks
    SB = min(512, S)      # s block (matmul moving free dim)
    NSB = S // SB         # s blocks
    NST = S // P          # s tiles
    EB = min(512, H)      # free-dim block for weight streaming
    NEB = H // EB
    DG = EB // P          # d tiles per streamed weight block
    scale = 1.0 / max(math.sqrt(128.0), 1e-8)

    hsT, atT = io["hsT"], io["atT"]
    wqT, wkT, wvT, woT = io["wqT"], io["wkT"], io["wvT"], io["woT"]
    maskb, out = io["maskb"], io["out"]

    # ---- pools ----
    with (
        tc.tile_pool(name="big", bufs=1) as big,     # WqT slot, reused by ctxT
        tc.tile_pool(name="res", bufs=1) as res,     # resident activations
        tc.tile_pool(name="atp", bufs=1) as atp,     # atT slot, reused by LN spill
        tc.tile_pool(name="wst", bufs=3) as wst,     # streamed Wk/Wv chunks
        tc.tile_pool(name="sml", bufs=4) as sml,     # exp/LN intermediates
        tc.tile_pool(name="psum", bufs=8, space="PSUM") as psum,
    ):
        # ---- loads needed first (K/V proj); hs/wq DMAs deferred below.
        # The very first weight group + first at piece lead the queue so the
        # first K matmul can issue as soon as possible.
        CG = 4
        wk_g0 = wst.tile([P, CG, EB], BF16, tag="wkv", name="wk_g0")
        nc.sync.dma_start(
            wk_g0, wkT[0:CG * P, 0:EB].rearrange("(c p) d -> p c d", p=P))
        at_sb = atp.tile([P, C, T], BF16, tag="at_dsb")
        w4 = C // 4
        nc.sync.dma_start(
            at_sb[:, 0:w4, :],
            atT[0:w4 * P, :].rearrange("(c p) t -> p c t", p=P))
        mb_sb = res.tile([P, NT], F32, tag="maskb")
        nc.sync.dma_start(mb_sb, maskb)
        for cg4 in range(1, 4):
            nc.sync.dma_start(
                at_sb[:, cg4 * w4:(cg4 + 1) * w4, :],
                atT[cg4 * w4 * P:(cg4 + 1) * w4 * P, :].rearrange(
                    "(c p) t -> p c t", p=P))

        ones_col = res.tile([P, 1], BF16, tag="ones_col")
        nc.vector.memset(ones_col, 1.0)
        inv_rs2 = (EPS_LN / (rs * rs)) if rs > 0 else 3.0e38
        eps_sb = res.tile([P, 1], F32, tag="eps")
        nc.vector.memset(eps_sb, inv_rs2)

        if use_qkv_bias:
            bq_sb = res.tile([P, C], F32, tag="bq")
            nc.sync.dma_start(bq_sb, io["bq"].rearrange("(c p) -> p c", p=P))
            bk_sb = res.tile([P, C], F32, tag="bk")
            nc.sync.dma_start(bk_sb, io["bk"].rearrange("(c p) -> p c", p=P))
            bv_sb = res.tile([P, H], F32, tag="bv")
            nc.gpsimd.dma_start(bv_sb, _bcast_row_ap(io["bv"], P))
            bo_sb = res.tile([P, H], F32, tag="bo")
            nc.gpsimd.dma_start(bo_sb, _bcast_row_ap(io["bo"], P))
        if use_gamma_beta:
            gm_sb = res.tile([P, H], F32, tag="gamma")
            nc.gpsimd.dma_start(gm_sb, _bcast_row_ap(io["gamma"], P))
            bt_sb = res.tile([P, H], F32, tag="beta")
            nc.gpsimd.dma_start(bt_sb, _bcast_row_ap(io["beta"], P))

        # ---- K projection: kT_sb[p, h, t] = K^T (d on partitions) ----
        # Weight chunks are DMA'd in groups of CG c-chunks (one 3D-AP DMA)
        # to amortize per-DMA fixed costs; a [128,512]bf16 chunk otherwise
        # costs more DMA time than the matmuls it feeds.
        kT_sb = res.tile([P, C, T], BF16, tag="kT")
        for dg in range(C // DG):
            kps = [psum.tile([P, T], F32, tag="ps", name=f"kps{dl}")
                   for dl in range(DG)]
            for cg in range(C // CG):
                if dg == 0 and cg == 0:
                    wk_g = wk_g0
                else:
                    wk_g = wst.tile([P, CG, EB], BF16, tag="wkv")
                    nc.sync.dma_start(
                        wk_g,
                        wkT[cg * CG * P:(cg + 1) * CG * P,
                            dg * EB:(dg + 1) * EB].rearrange(
                                "(c p) d -> p c d", p=P))
                for cl in range(CG):
                    c = cg * CG + cl
                    for dl in range(DG):
                        nc.tensor.matmul(
                            kps[dl], wk_g[:, cl, dl * P:(dl + 1) * P],
                            at_sb[:, c, :],
                            start=(c == 0), stop=(c == C - 1),
                        )
            for dl in range(DG):
                d = dg * DG + dl
                if use_qkv_bias:
                    nc.scalar.activation(kT_sb[:, d, :], kps[dl], AF.Identity,
                                         bias=bk_sb[:, d:d + 1])
                else:
                    nc.scalar.copy(kT_sb[:, d, :], kps[dl])

        # ---- V projection: v_sb[p, tt, d] = V (t on partitions) ----
        v_sb = res.tile([P, NT, H], BF16, tag="v")
        for eb in range(NEB):
            vps = [psum.tile([P, EB], F32, tag="ps", name=f"vps{tt}")
                   for tt in range(NT)]
            for cg in range(C // CG):
                wv_g = wst.tile([P, CG, EB], BF16, tag="wkv")
                nc.sync.dma_start(
                    wv_g,
                    wvT[cg * CG * P:(cg + 1) * CG * P,
                        eb * EB:(eb + 1) * EB].rearrange(
                            "(c p) d -> p c d", p=P))
                for cl in range(CG):
                    c = cg * CG + cl
                    for tt in range(NT):
                        nc.tensor.matmul(
                            vps[tt], at_sb[:, c, tt * P:(tt + 1) * P],
                            wv_g[:, cl, :],
                            start=(c == 0), stop=(c == C - 1),
                        )
            for tt in range(NT):
                nc.scalar.copy(v_sb[:, tt, eb * EB:(eb + 1) * EB], vps[tt])
        if use_qkv_bias:
            for tt in range(NT):
                nc.vector.tensor_add(v_sb[:, tt, :], v_sb[:, tt, :], bv_sb)

        # ---- deferred resident loads (only Q proj needs them); emitted
        # after the K/V weight streams so those win the DMA pipe early.
        # Small pieces, ordered so Q-proj's first tiles (low d, low s-block)
        # land first: wq by d-block, hs by s-block. All pieces beyond the
        # ones Q-proj touches first are release-gated (add_dep_helper below)
        # on Q-proj's own matmul progress, so they never steal HBM bandwidth
        # from the K/V weight streams.
        NDB = H // 512
        hs_sb = res.tile([P, C, S], BF16, tag="hsT")
        wq_sb = big.tile([P, C, H], BF16, tag="big")
        wq_dmas = {b: [] for b in range(NDB)}
        hs_dmas = {b: [] for b in range(NSB)}
        for dblk in range(NDB):
            for c in range(C):
                i = nc.sync.dma_start(
                    wq_sb[:, c, dblk * 512:(dblk + 1) * 512],
                    wqT[c * P:(c + 1) * P, dblk * 512:(dblk + 1) * 512])
                wq_dmas[dblk].append(i)
        for sbb in range(NSB):
            for c in range(C):
                i = nc.sync.dma_start(
                    hs_sb[:, c, sbb * SB:(sbb + 1) * SB],
                    hsT[c * P:(c + 1) * P, sbb * SB:(sbb + 1) * SB])
                hs_dmas[sbb].append(i)

        # ---- Q projection: qT_sb[p, h, s] = Q^T (WqT resident) ----
        qT_sb = res.tile([P, C, S], BF16, tag="qT")
        q_last_mm = {}
        for sb in range(NSB):
            for d in range(C):
                ps = psum.tile([P, SB], F32, tag="ps", name="qps")
                for c in range(C):
                    mm = nc.tensor.matmul(
                        ps, wq_sb[:, c, d * P:(d + 1) * P],
                        hs_sb[:, c, sb * SB:(sb + 1) * SB],
                        start=(c == 0), stop=(c == C - 1),
                    )
                q_last_mm[(sb, d)] = mm
                if use_qkv_bias:
                    nc.scalar.activation(qT_sb[:, d, sb * SB:(sb + 1) * SB],
                                         ps, AF.Identity,
                                         bias=bq_sb[:, d:d + 1])
                else:
                    nc.scalar.copy(qT_sb[:, d, sb * SB:(sb + 1) * SB], ps)
        # release-gates: wq dblk b (b>=1) after Q(sb0, d=4b-4) finishes;
        # hs sb-block 1 after Q(sb0, d=1).
        from concourse.bass import _add_dep_helper
        for b in range(1, NDB):
            gate = q_last_mm[(0, max(4 * b - 4, 0))]
            for dma in wq_dmas[b]:
                _add_dep_helper(dma.ins, gate.ins, sync=True,
                                reason="pace wq stream behind Q progress")
        for b in range(1, NSB):
            gate = q_last_mm[(0, 1)]
            for dma in hs_dmas[b]:
                _add_dep_helper(dma.ins, gate.ins, sync=True,
                                reason="pace hs stream behind Q progress")

        # ---- attention -> ctxT (reuses the WqT SBUF slot) ----
        # Per head, all PE work (scores, den, ctx) is emitted contiguously so
        # the PE stream never waits on the DVE reciprocal; the [1,s] -> [128,s]
        # reciprocal broadcast rides the DMA engines (partition-stride-0 read)
        # instead of a K=1 fp32 matmul.
        ctx_sb = big.tile([P, C, S], BF16, tag="big")

        def attn_tail(prev):
            # den/ctx (PE) + normalize (DVE/Pool) for an already-exp'd pair
            s_sl, pair, exps = prev
            for h in pair:
                ps_den = psum.tile([1, SB], F32, tag="ps", name="den")
                for tt in range(NT):
                    nc.tensor.matmul(ps_den, ones_col, exps[h][tt],
                                     start=(tt == 0), stop=(tt == NT - 1))
                ps_c = psum.tile([P, SB], F32, tag="ps", name="ctxT")
                for tt in range(NT):
                    nc.tensor.matmul(
                        ps_c, v_sb[:, tt, h * P:(h + 1) * P], exps[h][tt],
                        start=(tt == 0), stop=(tt == NT - 1),
                    )
                rcp = sml.tile([1, SB], F32, tag="rcp", bufs=2)
                nc.vector.reciprocal(rcp, ps_den)
                bc_sb = sml.tile([P, SB], F32, tag="bcast_sb", bufs=2)
                nc.gpsimd.partition_broadcast(bc_sb, rcp)
                nc.vector.tensor_mul(ctx_sb[:, h, s_sl], ps_c, bc_sb)

        # Software-pipelined across head pairs: pair p's scores (PE) are
        # emitted before pair p-1's den/ctx, so the PE always has exp-
        # independent work in flight while ACT computes pair p's exps.
        # PSUM: 4 score banks (pair p) + 4 den/ctx banks (pair p-1) == 8.
        prev = None
        for sb in range(NSB):
            s_sl = slice(sb * SB, (sb + 1) * SB)
            for h0 in range(0, NH, 2):
                pair = (h0, h0 + 1)
                exps = {}
                for h in pair:
                    exps[h] = []
                    for tt in range(NT):
                        ps_s = psum.tile([P, SB], F32, tag="ps",
                                         name="scoresT")
                        nc.tensor.matmul(
                            ps_s, kT_sb[:, h, tt * P:(tt + 1) * P],
                            qT_sb[:, h, s_sl], start=True, stop=True,
                        )
                        e_sb = sml.tile([P, SB], BF16, tag="expT", bufs=8)
                        nc.scalar.activation(e_sb, ps_s, AF.Exp,
                                             bias=mb_sb[:, tt:tt + 1],
                                             scale=scale)
                        exps[h].append(e_sb)
                if prev is not None:
                    attn_tail(prev)
                prev = (s_sl, pair, exps)
        attn_tail(prev)

        # ---- out projection + LayerNorm, [s, e] layout ----
        # SG4 s-tiles per group x NEBP passes over disjoint e-halves: each
        # pass holds SG4*EBL PSUM banks and streams only its e-half of WoT,
        # halving WoT traffic vs a full-row stream. Deltas spill to the dead
        # hsT slot (exact size match) and LayerNorm runs from SBUF.
        SG4 = min(4, NST)
        NEBP = 2 if NEB >= 2 else 1
        EBL = NEB // NEBP
        EW = EBL * EB  # e-width per pass

        def emit_pass(sg4, ebp, si_list, d_big, gate):
            dps = {(si, ebl): psum.tile([P, EB], F32, tag="ps",
                                        name=f"delta{si}_{ebl}")
                   for si in si_list for ebl in range(EBL)}
            for cp in range(C // 2):
                # c-pair chunks (one DMA) reusing the dead wkv slots
                wo_t = wst.tile([P, 2, EW], BF16, tag="wo", name="wo_t", bufs=5)
                wo_dma = nc.sync.dma_start(
                    wo_t,
                    woT[cp * 2 * P:(cp + 1) * 2 * P,
                        ebp * EW:(ebp + 1) * EW].rearrange(
                            "(c p) e -> p c e", p=P))
                if gate is not None:
                    # don't let the Wo prefetch steal HBM bandwidth from
                    # the Q-proj weight stream; attention has slack
                    _add_dep_helper(wo_dma.ins, gate.ins, sync=True,
                                    reason="pace Wo prefetch behind Q")
                for cl in range(2):
                    c = cp * 2 + cl
                    for si in si_list:
                        st = sg4 * SG4 + si
                        for ebl in range(EBL):
                            nc.tensor.matmul(
                                dps[(si, ebl)],
                                ctx_sb[:, c, st * P:(st + 1) * P],
                                wo_t[:, cl, ebl * EB:(ebl + 1) * EB],
                                start=(c == 0), stop=(c == C - 1),
                            )
            for si in si_list:
                for ebl in range(EBL):
                    nc.scalar.copy(d_big[:, si, ebp * EBL + ebl, :],
                                   dps[(si, ebl)])

        def emit_ln(sg4, si, d_big):
                st = sg4 * SG4 + si
                if use_qkv_bias:
                    nc.vector.tensor_add(
                        d_big[:, si].rearrange("p a b -> p (a b)"),
                        d_big[:, si].rearrange("p a b -> p (a b)"), bo_sb)
                stats = sml.tile([P, NEB, 6], F32, tag="stats")
                for eb in range(NEB):
                    nc.vector.bn_stats(stats[:, eb, :], d_big[:, si, eb, :])
                mv = sml.tile([P, 2], F32, tag="mv")
                nc.vector.bn_aggr(mv, stats)
                sd = sml.tile([P, 1], F32, tag="sd")
                nc.scalar.activation(sd, mv[:, 1:2], AF.Sqrt, bias=eps_sb)
                rstd = sml.tile([P, 1], F32, tag="rstd")
                nc.vector.reciprocal(rstd, sd)
                for eb in range(NEB):
                    o_sb = sml.tile([P, EB], F32, tag="o_sb", bufs=2)
                    nc.vector.tensor_scalar(
                        o_sb, d_big[:, si, eb, :], mv[:, 0:1], rstd,
                        op0=ALU.subtract, op1=ALU.mult,
                    )
                    if use_gamma_beta:
                        nc.vector.tensor_mul(o_sb, o_sb,
                                             gm_sb[:, eb * EB:(eb + 1) * EB])
                        nc.vector.tensor_add(o_sb, o_sb,
                                             bt_sb[:, eb * EB:(eb + 1) * EB])
                    nc.sync.dma_start(
                        out[st * P:(st + 1) * P, eb * EB:(eb + 1) * EB], o_sb)

        n_sg4 = NST // SG4
        for sg4 in range(n_sg4):
            d_big = res.tile([P, SG4, NEB, EB], F32, tag="hsT",
                             name=f"dbig{sg4}")
            gate = (q_last_mm[(NSB - 1, C - 1)] if sg4 == 0 else None)
            last = (sg4 == n_sg4 - 1)
            if not last or NEBP == 1 or SG4 < 4:
                for ebp in range(NEBP):
                    emit_pass(sg4, ebp, list(range(SG4)), d_big,
                              gate if ebp == 0 else None)
                for si in range(SG4):
                    emit_ln(sg4, si, d_big)
            else:
                # Last group: split the final e-pass progressively finer so
                # each LayerNorm overlaps the next sub-pass's matmuls,
                # shrinking the kernel-exit tail to a single LN.
                emit_pass(sg4, 0, list(range(SG4)), d_big, gate)
                emit_pass(sg4, 1, [0, 1], d_big, None)
                emit_ln(sg4, 0, d_big)
                emit_pass(sg4, 1, [2, 3], d_big, None)
                emit_ln(sg4, 1, d_big)
                emit_ln(sg4, 2, d_big)
                emit_ln(sg4, 3, d_big)


def build_nc(S, T, H, NH, rs, use_qkv_bias=False, use_gamma_beta=False):
    nc = bacc.Bacc("TRN2", target_bir_lowering=False, debug=False)
    io = {
        "hsT": nc.dram_tensor("hsT", [H, S], BF16, kind="ExternalInput")[:],
        "atT": nc.dram_tensor("atT", [H, T], BF16, kind="ExternalInput")[:],
        "wqT": nc.dram_tensor("wqT", [H, H], BF16, kind="ExternalInput")[:],
        "wkT": nc.dram_tensor("wkT", [H, H], BF16, kind="ExternalInput")[:],
        "wvT": nc.dram_tensor("wvT", [H, H], BF16, kind="ExternalInput")[:],
        "woT": nc.dram_tensor("woT", [H, H], BF16, kind="ExternalInput")[:],
        "maskb": nc.dram_tensor("maskb", [P, T // P], F32,
                                kind="ExternalInput")[:],
        "out": nc.dram_tensor("out", [S, H], F32, kind="ExternalOutput")[:],
    }
    if use_qkv_bias:
        for n in ("bq", "bk", "bv", "bo"):
            io[n] = nc.dram_tensor(n, [H], F32, kind="ExternalInput")[:]
    if use_gamma_beta:
        for n in ("gamma", "beta"):
            io[n] = nc.dram_tensor(n, [H], F32, kind="ExternalInput")[:]
    with tile.TileContext(nc) as tc:
        emit_cross_attn(tc, io, S, T, H, NH, rs, use_qkv_bias, use_gamma_beta)
    nc.finalize()
    return nc


def _nan_clean(x, lim=10000.0):
    return np.nan_to_num(x, nan=0.0, posinf=lim, neginf=-lim)


def prepare_in_maps(hidden_states, audio_tokens, attention_mask,
                    Wq, bq, Wk, bk, Wv, bv, Wo, bo,
                    use_qkv_bias, n_cores=8):
    """Host-side shard + transpose + bf16 cast. Returns list of in_maps."""
    bf = ml_dtypes.bfloat16
    B, S_full, H = hidden_states.shape
    halves = n_cores // B
    Sc = S_full // halves
    hs = _nan_clean(np.asarray(hidden_states, np.float32))
    at = _nan_clean(np.asarray(audio_tokens, np.float32))
    wqT = np.ascontiguousarray(np.asarray(Wq, np.float32).T.astype(bf))
    wkT = np.ascontiguousarray(np.asarray(Wk, np.float32).T.astype(bf))
    wvT = np.ascontiguousarray(np.asarray(Wv, np.float32).T.astype(bf))
    woT = np.ascontiguousarray(np.asarray(Wo, np.float32).T.astype(bf))
    in_maps = []
    for i in range(n_cores):
        b, half = i // halves, i % halves
        hsT = np.ascontiguousarray(
            hs[b, half * Sc:(half + 1) * Sc, :].T.astype(bf))
        atT = np.ascontiguousarray(at[b].T.astype(bf))
        m = np.asarray(attention_mask[b])
        mbias = np.where(m > 0.5, 0.0, MASK_NEG).astype(np.float32)
        maskb = np.ascontiguousarray(mbias.reshape(-1, P).T)  # [P, T//P]
        im = {"hsT": hsT, "atT": atT, "wqT": wqT, "wkT": wkT,
              "wvT": wvT, "woT": woT, "maskb": maskb}
        if use_qkv_bias:
            im["bq"] = np.asarray(bq, np.float32)
            im["bk"] = np.asarray(bk, np.float32)
            im["bv"] = np.asarray(bv, np.float32)
            im["bo"] = np.asarray(bo, np.float32)
        in_maps.append(im)
    return in_maps


_NC_CACHE = {}


def get_nc(Sc, T, H, NH, rs, use_qkv_bias, use_gamma_beta):
    key = (Sc, T, H, NH, rs, use_qkv_bias, use_gamma_beta)
    if key not in _NC_CACHE:
        _NC_CACHE[key] = build_nc(Sc, T, H, NH, rs, use_qkv_bias,
                                  use_gamma_beta)
    return _NC_CACHE[key]


def kernel(hidden_states, audio_tokens, attention_mask,
           Wq, bq, Wk, bk, Wv, bv, Wo, bo, ln_gamma, ln_beta, residual_scale):
    from concourse.bass_utils import run_bass_kernel_spmd

    B, S_full, H = hidden_states.shape
    T = audio_tokens.shape[1]
    NH = 16
    n_cores = 8
    halves = n_cores // B
    Sc = S_full // halves
    rs = float(np.clip(np.float32(residual_scale), 0.0, RES_SCALE_MAX))
    use_qkv_bias = any(np.any(np.asarray(x) != 0) for x in (bq, bk, bv, bo))
    use_gamma_beta = (np.any(np.asarray(ln_gamma) != 1.0)
                      or np.any(np.asarray(ln_beta) != 0.0))

    nc = get_nc(Sc, T, H, NH, rs, use_qkv_bias, use_gamma_beta)
    in_maps = prepare_in_maps(hidden_states, audio_tokens, attention_mask,
                              Wq, bq, Wk, bk, Wv, bv, Wo, bo,
                              use_qkv_bias, n_cores)
    if use_gamma_beta:
        for im in in_maps:
            im["gamma"] = np.asarray(ln_gamma, np.float32)
            im["beta"] = np.asarray(ln_beta, np.float32)

    res = run_bass_kernel_spmd(nc, in_maps, core_ids=list(range(n_cores)))
    out = np.empty((B, S_full, H), np.float32)
    for i in range(n_cores):
        b, half = i // halves, i % halves
        out[b, half * Sc:(half + 1) * Sc, :] = res.results[i]["out"]
    return out

